# revision 71
# baseline (speedup 1.0000x reference)
"""Dilated MHSA block on 8 Trainium2 NeuronCores (v2).

Sharding: sequence-parallel. Core c (0..7) handles batch b=c//4, query chunk
[512*(c%4), 512*(c%4)+512). DILATION=2 splits tokens into two independent
parity classes; the host de-interleaves tokens by parity so the attention
becomes a dense +/-8 band per 256-query parity block (halo 8 per side).
Token columns per core: [272 even | 272 odd] (256 queries + 2*8 halo each).

v2 structure (HW-profiled rewrite of v1):
  - q/k projections run fp8(e4m3) DoubleRow (x scaled 16, weights 256;
    descale 1/4096 at eviction; QK-norm erases the uniform scale). The
    v projection stays f16 (fp8 v alone costs 3.7e-2 rel err).
  - Dual DMA queues: SP carries xT8+wqk+ow (+y out), ACT carries consts+
    xT+wv; host tensors pre-packed so every transfer moves >=1KB
    contiguous per partition line.
  - Emission order: q/k matmuls first (PE), then v slabs 0,1,2 (f16),
    k-norm selector matmuls, v slabs 3,4,5 — so the ACT/DVE/Pool norm
    chains hide under the v projection's PE stream, and attention on
    parity 0 can start while parity-1 v slabs are still projecting.
  - All reciprocals use the single-instruction reciprocal_approx_fast
    (~5x faster on HW than InstReciprocal, which the sim undermodels).
  - Norm chains: evict (ACT identity, fused bias+descale) -> square (DVE
    f16 2x) -> block-diag selector matmul (PE, broadcasts per-head sumsq
    over the head's rows) -> approx-reciprocal (DVE, from PSUM) -> sqrt
    (ACT, emits f16) -> normalize multiply (Pool, f16). k's 1/|k| folds
    into k_sb so attention needs no per-key scale.
  - Attention per (head, parity): 3 score matmuls over band windows
    A[0:144) B[112:256) C[240:256), exp FIRST (ACT, straight off PSUM:
    qk-normed scores lie in [-1,1] so unmasked exp is finite), then ONE
    f16 0/1-mask multiply on DVE (2x SBUF tier), 4 AV matmuls into a
    (128,512) PSUM pair tile whose rows 64-127 hold the denominator
    (64-wide ones block interleaved with v); ACT evicts, DVE
    approx-reciprocal, Pool multiplies.
  - Output projection accumulates online, lagged 2 pairs; v/out biases
    fold into one rank-1 row (ybr) added during y accumulation.
"""

import os
import sys

for _p in ("/opt/trn_rl_repo", "/root/.axon_site/_ro/trn_rl_repo"):
    if os.path.isdir(_p) and _p not in sys.path:
        sys.path.insert(0, _p)

import numpy as np

import concourse.bass as bass
import concourse.mybir as mybir
import concourse.tile as tile
from concourse import bacc
from concourse import bass_utils

F32 = mybir.dt.float32
FR = mybir.dt.float32r
F16 = mybir.dt.float16
F8 = mybir.dt.float8e4
DR = mybir.MatmulPerfMode.DoubleRow

B, N, D = 2, 2048, 1024
H, DH = 16, 64
NCORES = 8
CHUNK = 512          # queries per core
PC = 256             # queries per parity block
HP = 8               # halo per side in parity space (= KWIN*DIL/2)
PL = PC + 2 * HP     # 272 keys per parity block
LOCAL = 2 * PL       # 544 token columns per core
KT = D // 128        # 8 contraction tiles
VS = 2048            # v slab stride: 16 heads x (64 feats | 64 ones)
DS = 1.0 / 4096.0    # descale after fp8 products (x*16 by w*256)


def _emit(tc, T):
    nc = tc.nc
    AF = mybir.ActivationFunctionType
    OP = mybir.AluOpType
    # build-time truncation for phase benchmarking (harness never sets this)
    PH = {"noop": -1, "dma": 0, "v": 1, "qk": 2, "att": 3, "full": 4}[
        os.environ.get("KPHASE", "full")
    ]

    with tc.tile_pool(name="persist", bufs=1) as pp:
        # ---- persistent tiles ------------------------------------------
        xT8 = pp.tile([128, KT, LOCAL], F8)
        xT = pp.tile([128, KT, LOCAL], F16)
        xTt = pp.tile([128, KT, 48], F16)
        wv = pp.tile([128, KT, D], F16)
        ow = pp.tile([128, KT, D], F16)
        ones1 = pp.tile([1, 128], F16)
        ybr = pp.tile([1, D], F16)
        v_sb = pp.tile([128, 6, VS], F16)
        k_sb = pp.tile([128, 8, LOCAL], F16)
        qn_sb = pp.tile([128, 8, LOCAL], F16)
        outTn = pp.tile([128, 8, CHUNK], F16)
        sqk = pp.tile([128, 8, 2, PL], F16)
        qb2 = pp.tile([128, 16], F32)
        selw = pp.tile([128, 128], F16)
        eye = pp.tile([128, 128], F16)
        mABC = pp.tile([128, 304], F16)

        if PH == -1:
            nc.vector.memset(eye, 0.0)
            return

        if os.environ.get("KDBG"):
            nc.vector.memset(v_sb, 0.0)
            nc.vector.memset(qn_sb, 0.0)

        # warm the ACT function tables before the queue fills with DMAs so
        # no LoadActFuncSet lands in front of the first real sqrt/exp
        warm = pp.tile([1, 2], F32)
        nc.vector.memset(warm, 1.0)
        nc.scalar.activation(warm, warm, AF.Sqrt)
        nc.scalar.activation(warm, warm, AF.Exp)

        # ---- DMAs up front, consumption order, two HWDGE queues --------
        # SP queue: qk inputs first (xT8 split per kt-pair so the first
        # q matmul starts ~1.2us in), then out-proj weights, y comes last.
        wq_tiles = [
            pp.tile([128, KT, 256], F8, name=f"wt{mp}") for mp in range(8)
        ]
        nc.sync.dma_start(xT8[:, 0:2], T["xT8"][:, 0:2])
        nc.sync.dma_start(wq_tiles[0], T["wqk"][:, 0])
        for k2 in range(2, KT, 2):
            nc.sync.dma_start(xT8[:, k2 : k2 + 2], T["xT8"][:, k2 : k2 + 2])
        for mp in range(1, 8):
            nc.sync.dma_start(wq_tiles[mp], T["wqk"][:, mp])
        nc.sync.dma_start(ow, T["ow"])
        # ones via Pool memset: Pool is idle until ~8us, and the strided
        # layout (96 runs of 128B per partition) is descriptor-heavy as DMA
        nc.gpsimd.memset(
            v_sb.rearrange("p m (h c) -> p m h c", c=128)[:, :, :, 64:128], 1.0
        )
        # ACT queue: small consts (needed by the early norm chains), then
        # the v-projection inputs (ACT's own compute starts ~10us in, after
        # the first q sqrt becomes ready, so these transfers hide)
        nc.scalar.dma_start(ones1, T["ones1"])
        nc.scalar.dma_start(ybr, T["ybr"])
        nc.scalar.dma_start(qb2, T["qb2"].rearrange("(m p) -> p m", p=128))
        nc.scalar.dma_start(selw, T["selw"])
        nc.scalar.dma_start(eye, T["eye"])
        nc.scalar.dma_start(mABC, T["maskABC"])
        # v inputs on the ACT queue: its first compute (q sqrt) is only
        # ready ~10us in, exactly when these transfers finish
        nc.scalar.dma_start(xTt, T["xTt"])
        nc.scalar.dma_start(xT, T["xT"])
        nc.scalar.dma_start(wv, T["wv"])

        if PH == 0:
            return

        def vap(m, h, p0, np_):
            """AV lhsT: [64 v-feats | 64 ones] of head h in slab m."""
            return v_sb[p0 : p0 + np_, m, 128 * h : 128 * h + 128]

        # ---- v projection (token-major, f16) ----------------------------
        # slabs: 0 e[0:128) 1 e[128:256) 3 o[0:128) 4 o[128:256); both
        # 16-token tails ride ONE combined slab "t" (e-tail rows 0:16,
        # o-tail rows 16:32 via the host-packed xTt) that evicts into the
        # v_sb slots 2 and 5 — halves the tail matmul count
        def v_evict_rows(vt, M, r0, slot):
            base = v_sb[0:M]
            for nh in range(2):
                dstv = bass.AP(
                    tensor=base.tensor,
                    offset=base.offset + slot * VS + nh * 1024,
                    ap=[list(base.ap[0]), [128, 8], [1, 64]],
                )
                src = vt[r0 : r0 + M, 512 * nh : 512 * (nh + 1)].rearrange(
                    "p (h c) -> p h c", c=64
                )
                if nh == 0:
                    nc.scalar.activation(dstv, src, AF.Copy)
                else:
                    nc.vector.tensor_copy(dstv, src)

        def v_slab(vps, m):
            if m == "t":
                vt = vps.tile([128, D], F32, tag="vp", name="vpt")
                for nh in range(2):
                    for kt in range(KT):
                        nc.tensor.matmul(
                            vt[0:48, 512 * nh : 512 * (nh + 1)],
                            xTt[:, kt],
                            wv[:, kt, 512 * nh : 512 * (nh + 1)],
                            start=(kt == 0),
                            stop=(kt == KT - 1),
                        )
                # o-tail sits at rows 32:48 (PSUM reads must start on a
                # 32-aligned partition base; rows 16:32 are zero padding)
                v_evict_rows(vt, 16, 0, 2)
                v_evict_rows(vt, 16, 32, 5)
                return
            c0 = {0: 0, 1: 128, 3: PL, 4: PL + 128}[m]
            vt = vps.tile([128, D], F32, tag="vp", name=f"vp{m}")
            for nh in range(2):
                for kt in range(KT):
                    nc.tensor.matmul(
                        vt[:, 512 * nh : 512 * (nh + 1)],
                        xT[:, kt, c0 : c0 + 128],
                        wv[:, kt, 512 * nh : 512 * (nh + 1)],
                        start=(kt == 0),
                        stop=(kt == KT - 1),
                    )
            v_evict_rows(vt, 128, 0, m)

        # ---- attention unit --------------------------------------------
        # software pipeline: unit k's AV matmuls are emitted AFTER unit
        # k+1's score matmuls, so each exp has a full unit of PE time to
        # complete before the PE queue reaches its consumer
        ot2_box = [None]
        den_box = []
        av_pend = [None]

        def att(stp, otp, pabp, rrp, h, p):
            st8 = att_scores(stp, pabp, h, p)
            if av_pend[0] is not None:
                att_av(otp, rrp, av_pend[0])
            av_pend[0] = st8

        def att_drain(otp, rrp):
            if av_pend[0] is not None:
                att_av(otp, rrp, av_pend[0])
                av_pend[0] = None

        def flush_dens(keep):
            while len(den_box) > keep:
                den_box.pop(0)()

        def att_scores(stp, pabp, h, p):
            g, a = h // 2, h % 2
            q0 = HP + PL * p
            kx = k_sb[64 * a : 64 * a + 64]
            qx = qn_sb[64 * a : 64 * a + 64]
            # mask prefill: st = maskABC via an identity matmul, scores
            # accumulate on top; masked entries exp to exactly 0, so the
            # whole unit needs ONE exp and no mask multiply at all
            st = stp.tile([128, 304], F32, tag="st", name="st")
            nc.tensor.matmul(st, eye, mABC, start=True, stop=True)
            nc.tensor.matmul(
                st[:, 0:144],
                kx[:, g, PL * p : PL * p + 128],
                qx[:, g, q0 : q0 + 144],
                start=False, stop=True, skip_group_check=True,
            )
            nc.tensor.matmul(
                st[:, 144:288],
                kx[:, g, PL * p + 128 : PL * p + 256],
                qx[:, g, q0 + 112 : q0 + 256],
                start=False, stop=True, skip_group_check=True,
            )
            nc.tensor.matmul(
                st[0:16, 288:304],
                kx[:, g, PL * p + 256 : PL * p + 272],
                qx[:, g, q0 + 240 : q0 + 256],
                start=False, stop=True, skip_group_check=True,
            )
            pab = pabp.tile([128, 304], F16, tag="pab", name="pab")
            nc.scalar.activation(pab, st, AF.Exp)
            return (pab, h, p)

        def att_av(otp, rrp, state):
            pab, h, p = state
            g, a = h // 2, h % 2
            s_p0 = 0 if p == 0 else 3
            s_p1 = 1 if p == 0 else 4
            if a == 0:
                ot2_box[0] = otp.tile([128, 512], F32, tag="ot", name="ot")
            ot = ot2_box[0][:, 256 * a : 256 * (a + 1)]
            nc.tensor.matmul(
                ot[:, 0:128], vap(s_p0, h, 0, 128), pab[:, 0:128],
                start=True, stop=False,
            )
            nc.tensor.matmul(
                ot[:, 112:128], vap(s_p1, h, 0, 128), pab[:, 144:160],
                start=False, stop=False,
            )
            nc.tensor.matmul(
                ot[:, 128:256], vap(s_p1, h, 0, 128), pab[:, 160:288],
                start=True, stop=False, skip_group_check=True,
            )
            nc.tensor.matmul(
                ot[:, 240:256], vap(2 if p == 0 else 5, h, 0, 16),
                pab[0:16, 288:304],
                start=False, stop=True,
            )
            if a == 1:
                # the denominator chain is DEFERRED one unit-pair (den_box
                # flushed by the caller) so exps never queue behind it
                ot2 = ot2_box[0]

                def den(ot2=ot2, g=g, p=p):
                    # reciprocal_approx_fast silently corrupts at partition
                    # base != 0 on HW, so the denominator rows base-shift
                    # to 0 first (alternating ACT/DVE); the numerator
                    # multiplies then read PSUM rows 0:64 directly
                    den0 = rrp.tile([64, 512], F32, tag="den", name="den0")
                    if g % 2 == 0:
                        nc.scalar.activation(den0, ot2[64:128], AF.Copy)
                    else:
                        nc.vector.tensor_copy(den0, ot2[64:128])
                    rr = rrp.tile([64, 512], F32, tag="rr", name="rr")
                    nc.vector.reciprocal_approx_fast(out=rr, in_=den0)
                    for aa in range(2):
                        nc.vector.tensor_tensor(
                            outTn[64 * aa : 64 * aa + 64, g, PC * p : PC * (p + 1)],
                            ot2[0:64, 256 * aa : 256 * (aa + 1)],
                            rr[:, 256 * aa : 256 * (aa + 1)],
                            OP.mult,
                        )

                den_box.append(den)

        # ---- online output projection ----------------------------------
        yacc = {}

        def op_piece(ypp, qb, g):
            if PH < 4:
                return
            if qb not in yacc:
                yacc[qb] = [
                    ypp.tile([128, 512], F32, tag="yp", name=f"yp{qb}n{nh}")
                    for nh in range(2)
                ]
            for nh in range(2):
                nc.tensor.matmul(
                    yacc[qb][nh],
                    outTn[:, g, 128 * qb : 128 * qb + 128],
                    ow[:, g, 512 * nh : 512 * (nh + 1)],
                    start=(g == 0),
                    stop=False,
                )
                if g == KT - 1:
                    # bias row: yb = vb @ out_w.T + out_b (host-folded)
                    nc.tensor.matmul(
                        yacc[qb][nh],
                        ones1,
                        ybr[:, 512 * nh : 512 * (nh + 1)],
                        start=False,
                        stop=True,
                    )

        def op_evict(ysbp, qb):
            if PH < 4:
                return
            for nh in range(2):
                ysb = ysbp.tile([128, 512], F16, tag="ysb", name="ysb")
                if nh == 0:
                    nc.scalar.activation(ysb, yacc[qb][nh], AF.Copy)
                else:
                    nc.vector.tensor_copy(ysb, yacc[qb][nh])
                nc.sync.dma_start(
                    T["y"][128 * qb : 128 * (qb + 1), 512 * nh : 512 * (nh + 1)],
                    ysb,
                )
            del yacc[qb]

        # ---- phase QK + ATT + OUT --------------------------------------
        with (
            tc.tile_pool(name="sqp", bufs=6) as sqp,
            tc.tile_pool(name="rrp", bufs=6) as rrp,
            tc.tile_pool(name="ysbp", bufs=6) as ysbp,
            tc.tile_pool(name="pabp", bufs=6) as pabp,
        ):

            def q_slab(qkps, m):
                mp, mi = m // 2, m % 2
                wt = wq_tiles[mp]
                ps = qkps.tile([128, 2, 256], F32, tag="qk", name=f"q{m}")
                for hf in range(2):
                    for kt in range(0, KT, 2):
                        nc.tensor.matmul(
                            ps[:, hf],
                            wt[:, kt : kt + 2, 128 * mi : 128 * (mi + 1)],
                            xT8[:, kt : kt + 2,
                                HP + PL * hf : HP + PL * hf + 256],
                            start=(kt == 0),
                            stop=(kt == KT - 2),
                            skip_group_check=(hf == 1),
                            perf_mode=DR,
                        )
                return ps

            def q_evict(qkps, normps, m, ps):
                g = m % 8
                bias_col = qb2[:, m : m + 1]
                base = qn_sb[:, g]
                dstv = bass.AP(
                    tensor=base.tensor,
                    offset=base.offset + HP,
                    ap=[list(base.ap[0]), [PL, 2], [1, 256]],
                )
                nc.vector.tensor_scalar(
                    dstv, ps, DS, bias_col, OP.mult, OP.add
                )
                sq = sqp.tile([128, 2, 256], F16, tag="sq", name="sq")
                nc.gpsimd.tensor_tensor(sq, dstv, dstv, OP.mult)
                nps = normps.tile([128, 512], F32, tag="np", name="nps")
                nc.tensor.matmul(
                    nps,
                    selw,
                    sq.rearrange("p a b -> p (a b)"),
                    start=True, stop=True,
                )
                inv = sqp.tile([128, 512], F32, tag="inv", name="inv")
                nc.vector.reciprocal_approx_fast(out=inv, in_=nps)
                rpw = sqp.tile([128, 512], F16, tag="rpw", name="rpw")
                nc.scalar.activation(rpw, inv, AF.Sqrt)
                nc.gpsimd.tensor_tensor(
                    dstv, dstv, rpw.rearrange("p (a b) -> p a b", b=256),
                    OP.mult,
                )

            def k_slab(qkps, m):
                g = m % 8
                mp, mi = m // 2, m % 2
                wt = wq_tiles[mp]
                bias_col = qb2[:, m : m + 1]
                halves = []
                for par in range(2):
                    ps = qkps.tile([128, PL], F32, tag="qk", name=f"k{m}h{par}")
                    for kt in range(0, KT, 2):
                        nc.tensor.matmul(
                            ps,
                            wt[:, kt : kt + 2, 128 * mi : 128 * (mi + 1)],
                            xT8[:, kt : kt + 2, PL * par : PL * (par + 1)],
                            start=(kt == 0),
                            stop=(kt == KT - 2),
                            perf_mode=DR,
                        )
                    halves.append(ps)
                for par in range(2):
                    ps = halves[par]
                    dst = k_sb[:, g, PL * par : PL * (par + 1)]
                    if par == 0:
                        nc.vector.tensor_scalar(
                            dst, ps, DS, bias_col, OP.mult, OP.add
                        )
                    else:
                        nc.scalar.activation(
                            dst, ps, AF.Identity, bias=bias_col, scale=DS
                        )
                    nc.gpsimd.tensor_tensor(sqk[:, g, par], dst, dst, OP.mult)

            def k_norm(normps, g, par):
                """per-head 1/|k| broadcast over the head's 64 rows via the
                block-diag selector matmul, folded into k_sb."""
                npsk = normps.tile([128, PL], F32, tag="np", name="npsk")
                nc.tensor.matmul(
                    npsk, selw, sqk[:, g, par], start=True, stop=True
                )
                invk = sqp.tile([128, PL], F32, tag="inv", name="invk")
                nc.vector.reciprocal_approx_fast(out=invk, in_=npsk)
                rpk = sqp.tile([128, PL], F16, tag="rpk", name="rpk")
                nc.scalar.activation(rpk, invk, AF.Sqrt)
                dst = k_sb[:, g, PL * par : PL * (par + 1)]
                nc.gpsimd.tensor_tensor(dst, dst, rpk, OP.mult)

            # PSUM pool lifetimes are managed manually so the parity-1 v
            # slabs (vps) can overlap the first parity-0 attention units:
            # qkps(2)+normps(2)+vps(4) = 8 banks, then qkps/normps release
            # and stp(2)+otp(2) open alongside vps, then vps releases and
            # ypp(4) opens. Releases are dependency-based, not barriers.
            # vps lives on the right-side PSUM stack so it can outlive
            # qkps/normps (released mid-stream) and die before ypp opens
            vps = tc.alloc_tile_pool(name="vps", bufs=2, space="PSUM", side="right")
            qkps = tc.alloc_tile_pool(name="qkps", bufs=2, space="PSUM")
            normps = tc.alloc_tile_pool(name="normps", bufs=2, space="PSUM")
            pending = []
            for m in range(8):
                ps = q_slab(qkps, m)
                pending.append((m, ps))
                if len(pending) > 1:
                    q_evict(qkps, normps, *pending.pop(0))
            for m in range(8, 16):
                k_slab(qkps, m)
                if m == 8:
                    while pending:
                        q_evict(qkps, normps, *pending.pop(0))
            # parity-0 v slabs: their PE stream hides the q/k norm
            # chains running on ACT/DVE/Pool
            for m in (0, 1, "t"):
                v_slab(vps, m)
            # batched: ALL Sqrts complete before any attention exp, so
            # the ACT table never thrashes between Sqrt and Exp
            for g in range(8):
                for par in range(2):
                    k_norm(normps, g, par)
            normps.release()
            qkps.release()

            if PH <= 2:
                vps.release()
                return

            # parity-1 v slabs complete before attention (pool releases are
            # dependency-based, so no barrier — attention still overlaps
            # the trailing v evicts)
            for m in (3, 4):
                v_slab(vps, m)
            vps.release()
            # stp/otp at depth 3 (PE can run three units ahead of the
            # exp/den chains); ypp shrinks to 2 banks by running each query
            # block's pieces as one serialized batch after its parity drains
            # after the AV/den deferrals the ot tiles are the longest-
            # lived PSUM tenants; st tiles free early at the exp
            stp = tc.alloc_tile_pool(name="stp", bufs=2, space="PSUM")
            otp = tc.alloc_tile_pool(name="otp", bufs=4, space="PSUM")
            ypp = tc.alloc_tile_pool(name="ypp", bufs=2, space="PSUM")
            for g in range(8):
                att(stp, otp, pabp, rrp, 2 * g, 0)
                att(stp, otp, pabp, rrp, 2 * g + 1, 0)
                flush_dens(1)
            # hoist the first o-pair ahead of the e-piece batches so the PE
            # starts o-scores while ACT/DVE drain the e backlog
            att(stp, otp, pabp, rrp, 0, 1)
            att(stp, otp, pabp, rrp, 1, 1)
            att_drain(otp, rrp)
            flush_dens(1)
            for g in range(8):
                op_piece(ypp, 0, g)
            op_evict(ysbp, 0)
            for g in range(8):
                op_piece(ypp, 1, g)
            op_evict(ysbp, 1)
            for g in range(1, 8):
                att(stp, otp, pabp, rrp, 2 * g, 1)
                att(stp, otp, pabp, rrp, 2 * g + 1, 1)
                flush_dens(1)
            att_drain(otp, rrp)
            flush_dens(0)
            for g in range(8):
                op_piece(ypp, 2, g)
            op_evict(ysbp, 2)
            for g in range(8):
                op_piece(ypp, 3, g)
            op_evict(ysbp, 3)
            ypp.release()
            otp.release()
            stp.release()
            if os.environ.get("KDBG"):
                nc.sync.dma_start(T["qn_d"], qn_sb)
                nc.sync.dma_start(T["k_d"], k_sb)
                nc.sync.dma_start(T["v_d"], v_sb)
                nc.sync.dma_start(T["outTn_d"], outTn)


_PROGRAM = None


def _declare_io(nc):
    T = {}

    def inp(name, shape, dt=F32):
        T[name] = nc.dram_tensor(name, shape, dt, kind="ExternalInput").ap()

    inp("xT8", (128, KT, LOCAL), F8)
    inp("xT", (128, KT, LOCAL), F16)
    inp("xTt", (128, KT, 48), F16)
    inp("wqk", (128, 8, KT, 256), F8)
    inp("wv", (128, KT, D), F16)
    inp("ow", (128, KT, D), F16)
    inp("ones1", (1, 128), F16)
    inp("ybr", (1, D), F16)
    inp("qb2", (2 * D,), F32)
    inp("selw", (128, 128), F16)
    inp("eye", (128, 128), F16)
    inp("maskABC", (128, 304), F16)
    T["y"] = nc.dram_tensor("y", (CHUNK, D), F16, kind="ExternalOutput").ap()
    if os.environ.get("KDBG"):
        T["qn_d"] = nc.dram_tensor("qn_d", (128, 8, LOCAL), F16, kind="ExternalOutput").ap()
        T["k_d"] = nc.dram_tensor("k_d", (128, 8, LOCAL), F16, kind="ExternalOutput").ap()
        T["v_d"] = nc.dram_tensor("v_d", (128, 6, VS), F16, kind="ExternalOutput").ap()
        T["outTn_d"] = nc.dram_tensor("outTn_d", (128, 8, CHUNK), F16, kind="ExternalOutput").ap()
    return T


def _build_program():
    global _PROGRAM
    if _PROGRAM is not None:
        return _PROGRAM
    nc = bacc.Bacc(
        "TRN2",
        target_bir_lowering=False,
        debug=False,
        enable_asserts=False,
        num_devices=NCORES,
    )
    T = _declare_io(nc)
    with tile.TileContext(nc) as tc:
        with nc.allow_low_precision(reason="fp16/fp8 matmul pipeline"):
            _emit(tc, T)
    nc.compile()
    _PROGRAM = nc
    return nc


NEG = -30000.0


def _host_masks(c0):
    """Additive mask (128, 304) = [A qq 0:144 | B qq 112:256 | C qq
    240:256 on rows 0:16], in parity space (shared between parities),
    prefilled into PSUM via an identity matmul. Band: live iff kk in
    [qq, qq+16] in parity-block coordinates and the token is in range.
    Rows 16:128 of the C columns are NEG so their exp is exactly 0."""
    def alive(kk):
        te = c0 - 16 + 2 * kk
        to = c0 - 15 + 2 * kk
        return (te >= 0) & (te < N) & (to >= 0) & (to < N)

    kkA = np.arange(128)[:, None]
    qqA = np.arange(144)[None, :]
    mA = np.where((qqA <= kkA) & (kkA <= qqA + 16) & alive(kkA), 0.0, NEG)

    kkB = 128 + np.arange(128)[:, None]
    qqB = 112 + np.arange(144)[None, :]
    mB = np.where((qqB <= kkB) & (kkB <= qqB + 16) & alive(kkB), 0.0, NEG)

    mCf = np.full((128, 16), NEG)
    kkC = 256 + np.arange(16)[:, None]
    qqC = 240 + np.arange(16)[None, :]
    mCf[0:16] = np.where((qqC <= kkC) & (kkC <= qqC + 16) & alive(kkC), 0.0, NEG)
    return np.concatenate([mA, mB, mCf], axis=1).astype(np.float16)


F8NP = mybir.dt.np(F8)


def _pack_kt(a):
    """(D, W) -> (128, KT, W): partition-major with kt pieces contiguous."""
    return np.ascontiguousarray(
        a.reshape(KT, 128, a.shape[1]).transpose(1, 0, 2)
    )


def _host_inputs(x, qkv_w, qkv_b, out_w, out_b):
    wqkT = qkv_w[: 2 * D].T.astype(np.float64) * 256.0  # (D, 2D)
    # (128, mp, kt, 256): per-(p, mp) the (kt, j) block is contiguous
    wqk = np.ascontiguousarray(
        wqkT.reshape(KT, 128, 8, 256).transpose(1, 2, 0, 3).astype(F8NP)
    )
    wv = _pack_kt(qkv_w[2 * D :].T.astype(np.float16))
    ow = _pack_kt(out_w.T.astype(np.float16))
    qb2 = np.ascontiguousarray(qkv_b[: 2 * D].astype(np.float32))
    # v-bias passes through softmax (weights sum to 1), so fold it with the
    # out-proj bias into one rank-1 row added during y accumulation.
    yb = (
        qkv_b[2 * D :].astype(np.float64) @ out_w.T.astype(np.float64)
        + out_b.astype(np.float64)
    )
    ybr = np.ascontiguousarray(yb.reshape(1, D).astype(np.float16))
    ones1 = np.ones((1, 128), dtype=np.float16)
    selw = np.zeros((128, 128), dtype=np.float16)
    selw[:64, :64] = 1.0
    selw[64:, 64:] = 1.0
    eye = np.eye(128, dtype=np.float16)

    in_maps = []
    for core in range(NCORES):
        b, i = divmod(core, 4)
        c0 = CHUNK * i
        xTc = np.zeros((D, LOCAL), dtype=np.float64)
        for par in range(2):
            t = c0 - 16 + par + 2 * np.arange(PL)
            ok = (t >= 0) & (t < N)
            blk = np.zeros((PL, D), dtype=np.float64)
            blk[ok] = x[b, t[ok]]
            xTc[:, PL * par : PL * (par + 1)] = blk.T
        xT8 = _pack_kt((xTc * 16.0).astype(F8NP))
        xT16 = _pack_kt(xTc.astype(np.float16))
        xTtc = np.zeros((D, 48))
        xTtc[:, 0:16] = xTc[:, 256:272]
        xTtc[:, 32:48] = xTc[:, PL + 256 : PL + 272]
        xTt = _pack_kt(xTtc.astype(np.float16))
        mABC = _host_masks(c0)
        in_maps.append(
            {
                "xT8": xT8,
                "xT": xT16,
                "xTt": xTt,
                "wqk": wqk,
                "wv": wv,
                "ow": ow,
                "ones1": ones1,
                "ybr": ybr,
                "qb2": qb2,
                "selw": selw,
                "eye": eye,
                "maskABC": mABC,
            }
        )
    return in_maps


def _unpermute(y_core):
    """Device y rows are [256 even queries | 256 odd]; interleave back."""
    out = np.empty((CHUNK, D), dtype=np.float32)
    out[0::2] = y_core[:PC]
    out[1::2] = y_core[PC:]
    return out


def kernel(x, qkv_w, qkv_b, out_w, out_b):
    x = np.asarray(x, dtype=np.float32)
    qkv_w = np.asarray(qkv_w, dtype=np.float32)
    qkv_b = np.asarray(qkv_b, dtype=np.float32)
    out_w = np.asarray(out_w, dtype=np.float32)
    out_b = np.asarray(out_b, dtype=np.float32)

    nc = _build_program()
    in_maps = _host_inputs(x, qkv_w, qkv_b, out_w, out_b)
    res = bass_utils.run_bass_kernel_spmd(nc, in_maps, core_ids=list(range(NCORES)))

    out = np.empty((B, N, D), dtype=np.float32)
    for core in range(NCORES):
        b, i = divmod(core, 4)
        out[b, CHUNK * i : CHUNK * (i + 1)] = _unpermute(
            np.asarray(res.results[core]["y"], dtype=np.float32)
        )
    return out
